# revision 6
# baseline (speedup 1.0000x reference)
"""MoE (63 routed experts top-7 + 1 shared expert) Trainium2 kernel.

Strategy: expert-parallel sparse dispatch. The router (softmax + top-k,
~0.3% of FLOPs) runs on host; tokens are gathered expert-major into
fixed-capacity weight slots, which are distributed across 8 NeuronCores.
Each core runs an identical (SPMD) Bass program: for every slot, a
1280->1280 Linear + exact GELU + 1280->1280 Linear over 1024 tokens,
feature-major (features on partitions, tokens on the free dim) so weights
need no transpose and biases ride the activation unit's per-partition
bias port. Outputs are gathered and gate-weighted back on host in the
reference's exact accumulation order.
"""

import sys
import math

sys.path.insert(0, "/opt/trn_rl_repo")

import numpy as np

D = 1280          # model dim
I = 1280          # expert inter dim
EXPERTS = 63      # routed experts
TOPK = 7          # routed top-k
CAP = 1024        # tokens per weight slot
CHUNK = 512       # tokens per matmul (fp32 moving-operand max)
KT = D // 128     # 10 contraction tiles
NCORES = 8

MM_DTYPE = "f32r"   # "f32r" | "bf16"

_PROGRAM_CACHE = {}


# ----------------------------------------------------------------- router

def _route(x2d, wr, br):
    """f32 softmax + top-k, matching jax.nn.softmax / jax.lax.top_k."""
    logits = (x2d @ wr + br).astype(np.float32)
    logits -= logits.max(-1, keepdims=True)
    np.exp(logits, out=logits)
    aff = logits / logits.sum(-1, keepdims=True)
    idx = np.argsort(-aff, axis=-1, kind="stable")[:, :TOPK]
    vals = np.take_along_axis(aff, idx, axis=-1)
    return idx.astype(np.int32), vals.astype(np.float32)


def _build_plan(T, idx):
    """Pack (token, expert) pairs expert-major into CAP-token pieces, plus
    the shared expert's T tokens, into 8 cores x S slots."""
    flat = idx.ravel()
    order = np.argsort(flat, kind="stable")          # expert-major slot order
    tok_of = (order // TOPK).astype(np.int64)
    counts = np.bincount(flat, minlength=EXPERTS)
    offs = np.concatenate([[0], np.cumsum(counts)])

    pieces = []  # (kind, expert, a, b)  [a:b) into the expert-major order
    for e in range(EXPERTS):
        a, b = int(offs[e]), int(offs[e + 1])
        while a < b:
            n = min(CAP, b - a)
            pieces.append(("r", e, a, a + n))
            a += n

    n_shared_min = math.ceil(T / CAP)
    S = max(1, math.ceil((len(pieces) + n_shared_min) / NCORES))
    n_shared = NCORES * S - len(pieces)
    # split T shared tokens near-evenly over n_shared pieces (each <= CAP)
    base, rem = divmod(T, n_shared)
    assert base + (1 if rem else 0) <= CAP
    t0 = 0
    for j in range(n_shared):
        n = base + (1 if j < rem else 0)
        pieces.append(("s", -1, t0, t0 + n))
        t0 += n
    assert t0 == T and len(pieces) == NCORES * S
    return pieces, S, order, tok_of


# ----------------------------------------------------------- device program

def _build_program(S, M):
    import concourse.bass as bass
    import concourse.mybir as mybir
    import concourse.tile as tile
    from concourse import bacc

    f32 = mybir.dt.float32
    bf16 = mybir.dt.bfloat16
    in_dt = bf16 if MM_DTYPE == "bf16" else mybir.dt.float32r

    nc = bacc.Bacc("TRN2", target_bir_lowering=False, debug=False,
                   enable_asserts=False, num_devices=NCORES)
    xT = nc.dram_tensor("xT", [KT, 128, M], in_dt, kind="ExternalInput").ap()
    w1s = nc.dram_tensor("w1s", [S, KT, 128, KT, 128], in_dt, kind="ExternalInput").ap()
    w2s = nc.dram_tensor("w2s", [S, KT, 128, KT, 128], in_dt, kind="ExternalInput").ap()
    b1s = nc.dram_tensor("b1s", [S, 128, KT], f32, kind="ExternalInput").ap()
    b2s = nc.dram_tensor("b2s", [S, 128, KT], f32, kind="ExternalInput").ap()
    yT = nc.dram_tensor("yT", [KT, 128, M], f32, kind="ExternalOutput").ap()

    CPS = CAP // CHUNK  # chunks per slot
    Gelu = mybir.ActivationFunctionType.Gelu
    Ident = mybir.ActivationFunctionType.Identity

    def mm_ap(ap):
        return ap

    with tile.TileContext(nc) as tc:
        with (
            tc.tile_pool(name="xa", bufs=3) as xa,
            tc.tile_pool(name="w1p", bufs=3) as w1p,
            tc.tile_pool(name="w2p", bufs=3) as w2p,
            tc.tile_pool(name="hp", bufs=3) as hp,
            tc.tile_pool(name="yo", bufs=6) as yo,
            tc.tile_pool(name="bp", bufs=2) as bp,
            tc.tile_pool(name="ps", bufs=8, space="PSUM") as ps,
        ):
            for s in range(S):
                col0 = s * CAP
                b1t = bp.tile([128, KT], f32, tag="b1", name="b1t")
                nc.sync.dma_start(out=b1t[:, :], in_=b1s[s])
                b2t = bp.tile([128, KT], f32, tag="b2", name="b2t")
                nc.sync.dma_start(out=b2t[:, :], in_=b2s[s])

                xc = []
                for c in range(CPS):
                    xt = xa.tile([128, KT, CHUNK], in_dt, tag="x", name="xt")
                    for k in range(KT):
                        nc.sync.dma_start(
                            out=xt[:, k, :],
                            in_=xT[k, :, col0 + c * CHUNK: col0 + (c + 1) * CHUNK])
                    xc.append(xt)

                hc = [hp.tile([128, KT, CHUNK], in_dt, tag="h", name=f"h{c}")
                      for c in range(CPS)]

                # layer 1: h = gelu(x @ w1 + b1), feature-major
                for io in range(KT):
                    w1t = w1p.tile([128, KT, 128], in_dt, tag="w1", name="w1t")
                    nc.sync.dma_start(out=w1t[:, :, :], in_=w1s[s, io])
                    for c in range(CPS):
                        pt = ps.tile([128, CHUNK], f32, tag="ps", name="pt")
                        for k in range(KT):
                            nc.tensor.matmul(pt[:, :], mm_ap(w1t[:, k, :]),
                                             mm_ap(xc[c][:, k, :]),
                                             start=(k == 0), stop=(k == KT - 1))
                        nc.scalar.activation(hc[c][:, io, :], pt[:, :], Gelu,
                                             bias=b1t[:, io:io + 1])

                # layer 2: y = h @ w2 + b2
                for io in range(KT):
                    w2t = w2p.tile([128, KT, 128], in_dt, tag="w2", name="w2t")
                    nc.sync.dma_start(out=w2t[:, :, :], in_=w2s[s, io])
                    for c in range(CPS):
                        pt = ps.tile([128, CHUNK], f32, tag="ps", name="pt")
                        for k in range(KT):
                            nc.tensor.matmul(pt[:, :], mm_ap(w2t[:, k, :]),
                                             mm_ap(hc[c][:, k, :]),
                                             start=(k == 0), stop=(k == KT - 1))
                        yt = yo.tile([128, CHUNK], f32, tag="y", name="yt")
                        nc.scalar.activation(yt[:, :], pt[:, :], Ident,
                                             bias=b2t[:, io:io + 1])
                        nc.sync.dma_start(
                            out=yT[io, :, col0 + c * CHUNK: col0 + (c + 1) * CHUNK],
                            in_=yt[:, :])
    nc.compile()
    return nc


def _get_program(S, M):
    key = (S, M, MM_DTYPE)
    if key not in _PROGRAM_CACHE:
        _PROGRAM_CACHE[key] = _build_program(S, M)
    return _PROGRAM_CACHE[key]


# ------------------------------------------------------------------ kernel

def _np_dt():
    import ml_dtypes
    return ml_dtypes.bfloat16 if MM_DTYPE == "bf16" else np.float32


def _arrange_w(w):
    """[D, I] -> [io, p, ko, c] so each (slot, io) block DMAs contiguously
    into an SBUF tile laid out [partition, ko, col]."""
    return np.ascontiguousarray(
        w.reshape(KT, 128, KT, 128).transpose(2, 1, 0, 3))


def kernel(x, sw1, sb1, sw2, sb2, rw1, rb1, rw2, rb2, wr, br, _trace=False):
    from concourse.bass_utils import run_bass_kernel_spmd

    x = np.asarray(x, dtype=np.float32)
    B, Sq, _ = x.shape
    T = B * Sq
    xf = np.ascontiguousarray(x.reshape(T, D))

    idx, vals = _route(xf, np.asarray(wr, np.float32), np.asarray(br, np.float32))
    pieces, S, order, tok_of = _build_plan(T, idx)
    M = S * CAP
    dt = _np_dt()

    rw1 = np.asarray(rw1, np.float32); rw2 = np.asarray(rw2, np.float32)
    rb1 = np.asarray(rb1, np.float32); rb2 = np.asarray(rb2, np.float32)
    sw1 = np.asarray(sw1, np.float32); sw2 = np.asarray(sw2, np.float32)
    sb1 = np.asarray(sb1, np.float32); sb2 = np.asarray(sb2, np.float32)

    # pre-arranged weights, cached per id of the weight arrays
    w1a = [_arrange_w(rw1[e]).astype(dt) for e in range(EXPERTS)]
    w2a = [_arrange_w(rw2[e]).astype(dt) for e in range(EXPERTS)]
    sw1a = _arrange_w(sw1).astype(dt)
    sw2a = _arrange_w(sw2).astype(dt)
    b1a = [np.ascontiguousarray(rb1[e].reshape(KT, 128).T) for e in range(EXPERTS)]
    b2a = [np.ascontiguousarray(rb2[e].reshape(KT, 128).T) for e in range(EXPERTS)]
    sb1a = np.ascontiguousarray(sb1.reshape(KT, 128).T)
    sb2a = np.ascontiguousarray(sb2.reshape(KT, 128).T)

    xfT = np.ascontiguousarray(xf.T)  # [D, T]
    tok_r = tok_of  # token of each expert-major (token,k) pair

    in_maps = []
    for core in range(NCORES):
        xT_core = np.zeros((D, M), dtype=dt)
        w1_core = np.zeros((S, KT, 128, KT, 128), dtype=dt)
        w2_core = np.zeros((S, KT, 128, KT, 128), dtype=dt)
        b1_core = np.zeros((S, 128, KT), dtype=np.float32)
        b2_core = np.zeros((S, 128, KT), dtype=np.float32)
        for j in range(S):
            kind, e, a, b = pieces[core * S + j]
            toks = tok_r[a:b] if kind == "r" else np.arange(a, b)
            xT_core[:, j * CAP: j * CAP + (b - a)] = xfT[:, toks]
            if kind == "r":
                w1_core[j] = w1a[e]; w2_core[j] = w2a[e]
                b1_core[j] = b1a[e]; b2_core[j] = b2a[e]
            else:
                w1_core[j] = sw1a; w2_core[j] = sw2a
                b1_core[j] = sb1a; b2_core[j] = sb2a
        in_maps.append({
            "xT": xT_core.reshape(KT, 128, M),
            "w1s": w1_core, "w2s": w2_core,
            "b1s": b1_core, "b2s": b2_core,
        })

    nc = _get_program(S, M)
    res = run_bass_kernel_spmd(nc, in_maps, core_ids=list(range(NCORES)),
                               trace=_trace)
    kernel.last_result = res

    TK = T * TOPK
    gated = np.empty((TK, D), dtype=np.float32)   # expert-major rows
    shared_out = np.empty((T, D), dtype=np.float32)
    for core in range(NCORES):
        Y = res.results[core]["yT"].reshape(D, M)
        for j in range(S):
            kind, e, a, b = pieces[core * S + j]
            block = Y[:, j * CAP: j * CAP + (b - a)].T  # [n, D]
            if kind == "r":
                gated[a:b] = block
            else:
                shared_out[a:b] = block

    g = vals.ravel()[order].astype(np.float32)
    gated *= g[:, None]
    ord2 = np.argsort(tok_of, kind="stable")      # token-major, expert asc
    routed = gated[ord2].reshape(T, TOPK, D).sum(axis=1, dtype=np.float32)

    out = shared_out + routed + xf
    return out.reshape(B, Sq, D).astype(np.float32)


# revision 8
# speedup vs baseline: 1.0618x; 1.0618x over previous
"""MoE (63 routed experts top-7 + 1 shared expert) Trainium2 kernel.

Strategy: expert-parallel sparse dispatch. The router (softmax + top-k,
~0.3% of FLOPs) runs on host; tokens are gathered expert-major into
fixed-capacity weight slots, which are distributed across 8 NeuronCores.
Each core runs an identical (SPMD) Bass program: for every slot, a
1280->1280 Linear + exact GELU + 1280->1280 Linear over 1024 tokens,
feature-major (features on partitions, tokens on the free dim) so weights
need no transpose and biases ride the activation unit's per-partition
bias port. Outputs are gathered and gate-weighted back on host in the
reference's exact accumulation order.
"""

import os
import sys
import math

sys.path.insert(0, "/opt/trn_rl_repo")

import numpy as np

D = 1280          # model dim
I = 1280          # expert inter dim
EXPERTS = 63      # routed experts
TOPK = 7          # routed top-k
CAP = 1024        # tokens per weight slot
CHUNK = 512       # tokens per matmul (fp32 moving-operand max)
KT = D // 128     # 10 contraction tiles
NCORES = 8

MM_DTYPE = os.environ.get("MM_DTYPE", "f32r")   # "f32r" | "bf16"

_PROGRAM_CACHE = {}


# ----------------------------------------------------------------- router

def _route(x2d, wr, br):
    """f32 softmax + top-k, matching jax.nn.softmax / jax.lax.top_k."""
    logits = (x2d @ wr + br).astype(np.float32)
    logits -= logits.max(-1, keepdims=True)
    np.exp(logits, out=logits)
    aff = logits / logits.sum(-1, keepdims=True)
    idx = np.argsort(-aff, axis=-1, kind="stable")[:, :TOPK]
    vals = np.take_along_axis(aff, idx, axis=-1)
    return idx.astype(np.int32), vals.astype(np.float32)


def _build_plan(T, idx):
    """Pack (token, expert) pairs expert-major into CAP-token pieces, plus
    the shared expert's T tokens, into 8 cores x S slots."""
    flat = idx.ravel()
    order = np.argsort(flat, kind="stable")          # expert-major slot order
    tok_of = (order // TOPK).astype(np.int64)
    counts = np.bincount(flat, minlength=EXPERTS)
    offs = np.concatenate([[0], np.cumsum(counts)])

    pieces = []  # (kind, expert, a, b)  [a:b) into the expert-major order
    for e in range(EXPERTS):
        a, b = int(offs[e]), int(offs[e + 1])
        while a < b:
            n = min(CAP, b - a)
            pieces.append(("r", e, a, a + n))
            a += n

    n_shared_min = math.ceil(T / CAP)
    S = max(1, math.ceil((len(pieces) + n_shared_min) / NCORES))
    n_shared = NCORES * S - len(pieces)
    # split T shared tokens near-evenly over n_shared pieces (each <= CAP)
    base, rem = divmod(T, n_shared)
    assert base + (1 if rem else 0) <= CAP
    t0 = 0
    for j in range(n_shared):
        n = base + (1 if j < rem else 0)
        pieces.append(("s", -1, t0, t0 + n))
        t0 += n
    assert t0 == T and len(pieces) == NCORES * S
    return pieces, S, order, tok_of


# ----------------------------------------------------------- device program

def _build_program(S, M):
    import concourse.bass as bass
    import concourse.mybir as mybir
    import concourse.tile as tile
    from concourse import bacc

    f32 = mybir.dt.float32
    bf16 = mybir.dt.bfloat16
    in_dt = bf16 if MM_DTYPE == "bf16" else mybir.dt.float32r

    nc = bacc.Bacc("TRN2", target_bir_lowering=False, debug=False,
                   enable_asserts=False, num_devices=NCORES)
    xT = nc.dram_tensor("xT", [KT, 128, M], in_dt, kind="ExternalInput").ap()
    w1s = nc.dram_tensor("w1s", [S, KT, 128, KT, 128], in_dt, kind="ExternalInput").ap()
    w2s = nc.dram_tensor("w2s", [S, KT, 128, KT, 128], in_dt, kind="ExternalInput").ap()
    b1s = nc.dram_tensor("b1s", [S, 128, KT], f32, kind="ExternalInput").ap()
    b2s = nc.dram_tensor("b2s", [S, 128, KT], f32, kind="ExternalInput").ap()
    yT = nc.dram_tensor("yT", [KT, 128, M], f32, kind="ExternalOutput").ap()

    CPS = CAP // CHUNK  # chunks per slot
    Gelu = mybir.ActivationFunctionType.Gelu
    Ident = mybir.ActivationFunctionType.Identity

    def mm_ap(ap):
        return ap

    with tile.TileContext(nc) as tc:
        with (
            tc.tile_pool(name="xa", bufs=3) as xa,
            tc.tile_pool(name="w1p", bufs=3) as w1p,
            tc.tile_pool(name="w2p", bufs=3) as w2p,
            tc.tile_pool(name="hp", bufs=3) as hp,
            tc.tile_pool(name="yo", bufs=6) as yo,
            tc.tile_pool(name="bp", bufs=2) as bp,
            tc.tile_pool(name="ps", bufs=8, space="PSUM") as ps,
        ):
            for s in range(S):
                col0 = s * CAP
                b1t = bp.tile([128, KT], f32, tag="b1", name="b1t")
                nc.sync.dma_start(out=b1t[:, :], in_=b1s[s])
                b2t = bp.tile([128, KT], f32, tag="b2", name="b2t")
                nc.sync.dma_start(out=b2t[:, :], in_=b2s[s])

                xc = []
                for c in range(CPS):
                    xt = xa.tile([128, KT, CHUNK], in_dt, tag="x", name="xt")
                    for k in range(KT):
                        nc.sync.dma_start(
                            out=xt[:, k, :],
                            in_=xT[k, :, col0 + c * CHUNK: col0 + (c + 1) * CHUNK])
                    xc.append(xt)

                hc = [hp.tile([128, KT, CHUNK], in_dt, tag="h", name=f"h{c}")
                      for c in range(CPS)]

                # layer 1: h = gelu(x @ w1 + b1), feature-major
                for io in range(KT):
                    w1t = w1p.tile([128, KT, 128], in_dt, tag="w1", name="w1t")
                    nc.sync.dma_start(out=w1t[:, :, :], in_=w1s[s, io])
                    for c in range(CPS):
                        pt = ps.tile([128, CHUNK], f32, tag="ps", name="pt")
                        for k in range(KT):
                            nc.tensor.matmul(pt[:, :], mm_ap(w1t[:, k, :]),
                                             mm_ap(xc[c][:, k, :]),
                                             start=(k == 0), stop=(k == KT - 1))
                        nc.scalar.activation(hc[c][:, io, :], pt[:, :], Gelu,
                                             bias=b1t[:, io:io + 1])

                # layer 2: y = h @ w2 + b2
                for io in range(KT):
                    w2t = w2p.tile([128, KT, 128], in_dt, tag="w2", name="w2t")
                    nc.sync.dma_start(out=w2t[:, :, :], in_=w2s[s, io])
                    for c in range(CPS):
                        pt = ps.tile([128, CHUNK], f32, tag="ps", name="pt")
                        for k in range(KT):
                            nc.tensor.matmul(pt[:, :], mm_ap(w2t[:, k, :]),
                                             mm_ap(hc[c][:, k, :]),
                                             start=(k == 0), stop=(k == KT - 1))
                        yt = yo.tile([128, CHUNK], f32, tag="y", name="yt")
                        nc.scalar.activation(yt[:, :], pt[:, :], Ident,
                                             bias=b2t[:, io:io + 1])
                        nc.sync.dma_start(
                            out=yT[io, :, col0 + c * CHUNK: col0 + (c + 1) * CHUNK],
                            in_=yt[:, :])
    nc.compile()
    return nc


def _get_program(S, M):
    key = (S, M, MM_DTYPE)
    if key not in _PROGRAM_CACHE:
        _PROGRAM_CACHE[key] = _build_program(S, M)
    return _PROGRAM_CACHE[key]


# ------------------------------------------------------------------ kernel

def _np_dt():
    import ml_dtypes
    return ml_dtypes.bfloat16 if MM_DTYPE == "bf16" else np.float32


def _arrange_w(w):
    """[D, I] -> [io, p, ko, c] so each (slot, io) block DMAs contiguously
    into an SBUF tile laid out [partition, ko, col]."""
    return np.ascontiguousarray(
        w.reshape(KT, 128, KT, 128).transpose(2, 1, 0, 3))


def kernel(x, sw1, sb1, sw2, sb2, rw1, rb1, rw2, rb2, wr, br, _trace=False):
    from concourse.bass_utils import run_bass_kernel_spmd

    x = np.asarray(x, dtype=np.float32)
    B, Sq, _ = x.shape
    T = B * Sq
    xf = np.ascontiguousarray(x.reshape(T, D))

    idx, vals = _route(xf, np.asarray(wr, np.float32), np.asarray(br, np.float32))
    pieces, S, order, tok_of = _build_plan(T, idx)
    M = S * CAP
    dt = _np_dt()

    rw1 = np.asarray(rw1, np.float32); rw2 = np.asarray(rw2, np.float32)
    rb1 = np.asarray(rb1, np.float32); rb2 = np.asarray(rb2, np.float32)
    sw1 = np.asarray(sw1, np.float32); sw2 = np.asarray(sw2, np.float32)
    sb1 = np.asarray(sb1, np.float32); sb2 = np.asarray(sb2, np.float32)

    # pre-arranged weights, cached per id of the weight arrays
    w1a = [_arrange_w(rw1[e]).astype(dt) for e in range(EXPERTS)]
    w2a = [_arrange_w(rw2[e]).astype(dt) for e in range(EXPERTS)]
    sw1a = _arrange_w(sw1).astype(dt)
    sw2a = _arrange_w(sw2).astype(dt)
    b1a = [np.ascontiguousarray(rb1[e].reshape(KT, 128).T) for e in range(EXPERTS)]
    b2a = [np.ascontiguousarray(rb2[e].reshape(KT, 128).T) for e in range(EXPERTS)]
    sb1a = np.ascontiguousarray(sb1.reshape(KT, 128).T)
    sb2a = np.ascontiguousarray(sb2.reshape(KT, 128).T)

    xfT = np.ascontiguousarray(xf.T)  # [D, T]
    tok_r = tok_of  # token of each expert-major (token,k) pair

    in_maps = []
    for core in range(NCORES):
        xT_core = np.zeros((D, M), dtype=dt)
        w1_core = np.zeros((S, KT, 128, KT, 128), dtype=dt)
        w2_core = np.zeros((S, KT, 128, KT, 128), dtype=dt)
        b1_core = np.zeros((S, 128, KT), dtype=np.float32)
        b2_core = np.zeros((S, 128, KT), dtype=np.float32)
        for j in range(S):
            kind, e, a, b = pieces[core * S + j]
            toks = tok_r[a:b] if kind == "r" else np.arange(a, b)
            xT_core[:, j * CAP: j * CAP + (b - a)] = xfT[:, toks]
            if kind == "r":
                w1_core[j] = w1a[e]; w2_core[j] = w2a[e]
                b1_core[j] = b1a[e]; b2_core[j] = b2a[e]
            else:
                w1_core[j] = sw1a; w2_core[j] = sw2a
                b1_core[j] = sb1a; b2_core[j] = sb2a
        in_maps.append({
            "xT": xT_core.reshape(KT, 128, M),
            "w1s": w1_core, "w2s": w2_core,
            "b1s": b1_core, "b2s": b2_core,
        })

    nc = _get_program(S, M)
    res = run_bass_kernel_spmd(nc, in_maps, core_ids=list(range(NCORES)),
                               trace=_trace)
    kernel.last_result = res

    TK = T * TOPK
    gated = np.empty((TK, D), dtype=np.float32)   # expert-major rows
    shared_out = np.empty((T, D), dtype=np.float32)
    for core in range(NCORES):
        Y = res.results[core]["yT"].reshape(D, M)
        for j in range(S):
            kind, e, a, b = pieces[core * S + j]
            block = Y[:, j * CAP: j * CAP + (b - a)].T  # [n, D]
            if kind == "r":
                gated[a:b] = block
            else:
                shared_out[a:b] = block

    g = vals.ravel()[order].astype(np.float32)
    gated *= g[:, None]
    ord2 = np.argsort(tok_of, kind="stable")      # token-major, expert asc
    routed = gated[ord2].reshape(T, TOPK, D).sum(axis=1, dtype=np.float32)

    out = shared_out + routed + xf
    return out.reshape(B, Sq, D).astype(np.float32)
